# revision 6
# baseline (speedup 1.0000x reference)
"""Trainium2 Bass kernel for nn_CausePredictor (RGCN + pairwise MLP).

Sharding: data-parallel over the pairwise row index i: 8 cores x 25 rows,
replicated over B=4.  All per-core differences are encoded as input DATA,
so one SPMD program serves all cores.

Math (matching reference.py):
  h   = sum_k Ahat_k.T @ (x[b] @ basis_k) + x[b] @ root + bias
  u   = h @ W1a   (j term),  v = h @ W1c  (i term)
  h1[b,i,j,:] = u[b,j] + v[b,i] + T[pos(i,j)]
  out = sigmoid(Wp . relu(relu(h1) @ W2)) * mask

v2 performance structure (TimelineSim cost model):
  - GEMM2 runs FULLY in fp8 DoubleRow (8 cyc/col vs v1's 12): rh1 is
    split into an fp8 hi stream + an fp8 lo residual stream (hi+lo is
    ~fp16-exact), W2 is single fp8.  Modeled max-rel-err 1.51e-2.
  - All carriers are fp16 (not bf16) for mantissa headroom.
  - The T[pos] table is NOT pair-expanded on host.  A per-core column
    permutation of j (baked into xT/ahat data, inverted on unshard)
    puts the 35-wide diagonal T-band at fixed columns [165,200) for
    every core; the remaining 165 "rect" columns have a row-independent
    T value folded into u'' = u + TA0 (one tiny TT per (b,mc)).
  - a = relu(u'' + v) in fp16: per-row stock tensor_scalar (DVE 2x /
    Act activation / Pool TS); band: TT + one custom DVE op
    relu(Src0+Src1) with v broadcast.
  - hi = fp8 cast of a via gpsimd CASTING DMA (free engine-wise).
  - lo = a - hi via tensor_sub on DVE/Pool.
  - relu2 (PSUM->SBUF) mostly on Act; GEMM3 with stationary rh2 and
    moving wp (1-col outputs, ~free) as in v1.
"""

import sys

sys.path.insert(0, "/opt/trn_rl_repo")

import numpy as np

B, S, D, M, P = 4, 200, 300, 512, 100
NREL, MAXL = 9, 10
NCORES = 8
IPC = S // NCORES          # 25 rows of i per core
SC = S + IPC               # 225: u columns ++ the core's i-slice
RW = 165                   # rect columns [0,165)
BW = S - RW                # 35 band columns [165,200)
FPC = IPC * S              # 5000 pairs per (b, core)
NCOL = 50                  # output columns per b (100 pairs each)
NU = 13                    # 12x 2-row units + 1x 1-row unit per b
HA, HB = 12, 13            # row halves: units 0..5 / 6..12

_prog_cache = {}


def _rel_adj(s):
    ra = np.arange(s)[None, :] - np.arange(s)[:, None]
    for i in range(s):
        ra[i, i + 1:] = 1
        num = 1
        for o in range(i - 1, -1, -2):
            ra[i, o] = -num
            if o - 1 >= 0:
                ra[i, o - 1] = -num
            num += 1
        ra[i, :i] = np.maximum(ra[i, :i], -8)
    return ra


def _pack_k(w):
    """[K, N] -> [128, ceil(K/128)*N], K chunked onto partitions."""
    k, n = w.shape
    nch = (k + 127) // 128
    out = np.zeros((128, nch * n), np.float32)
    for c in range(nch):
        r = min(128, k - c * 128)
        out[:r, c * n: c * n + n] = w[c * 128: c * 128 + r]
    return out


# ---------------- custom DVE op ------------------------------------------
def _register_custom_ops():
    from concourse.dve_spec import Spec, Src0, Src1, C0, relu, lower
    from concourse import dve_ops as dvo
    from concourse.dve_uop import DveOpSpec

    if "RELU_TTS_ANT" in dvo.CUSTOM_DVE_SPECS:
        return next(o for o in dvo.OPS if o.name == "RELU_TTS_ANT")
    spec = Spec(
        body=relu(Src0 + Src1 + C0),
        reference=lambda in0, in1, s0, s1, imm2: np.maximum(
            in0.astype(np.float32) + in1 + s0, 0.0),
    )
    shas = {}
    for ver in ("v3", "v4"):
        s = DveOpSpec(name="RELU_TTS_ANT", opcode=0, uops=lower(spec, ver=ver),
                      rd1_en=True)
        shas[ver] = s.sha(ver)
    op = dvo.DveOp("RELU_TTS_ANT", spec, subdim=False, uops_sha=shas)
    dvo.OPS.append(op)
    dvo.CUSTOM_DVE_SPECS[op.name] = spec
    dvo._SUB_OPCODE_FOR_NAME[op.name] = dvo._CUSTOM_DVE_ROW_BASE + len(dvo.OPS) - 1
    return op


def _build_program():
    import ml_dtypes  # noqa: F401
    import concourse.tile as tile
    from concourse import bacc, mybir

    RELU_TTS = _register_custom_ops()

    f32 = mybir.dt.float32
    fp16 = mybir.dt.float16
    fp8 = mybir.dt.float8e4
    AF = mybir.ActivationFunctionType
    OP = mybir.AluOpType
    PM = mybir.MatmulPerfMode

    nc = bacc.Bacc()

    dxT = nc.declare_dram_parameter("xT", [D, B * SC], fp16, isOutput=False)
    dahat = nc.declare_dram_parameter("ahat", [128, 4 * SC], fp16, isOutput=False)
    dbasis = nc.declare_dram_parameter("basis", [128, 1800], fp16, isOutput=False)
    droot = nc.declare_dram_parameter("root", [128, 900], fp16, isOutput=False)
    dbias = nc.declare_dram_parameter("bias", [128, 3], f32, isOutput=False)
    dw1a = nc.declare_dram_parameter("w1a", [128, 1536], fp16, isOutput=False)
    dw1c = nc.declare_dram_parameter("w1c", [128, 1536], fp16, isOutput=False)
    dw28 = nc.declare_dram_parameter("w28", [128, 2048], fp8, isOutput=False)
    dwp = nc.declare_dram_parameter("wp", [128, 4], fp16, isOutput=False)
    dta0 = nc.declare_dram_parameter("ta0", [128, 4 * S], fp16, isOutput=False)
    dtab = nc.declare_dram_parameter("tab", [128, 4 * IPC * BW], fp16, isOutput=False)
    dout = nc.declare_dram_parameter("out", [B * 100, NCOL], f32, isOutput=True)

    DCW = [128, 128, 44]   # D=300 chunks
    JCW = [128, 72]        # S=200 chunks

    # ---- engine assignment policies (tuning knobs) ----
    # a-row engine by linear index (b, mc, half, r)
    def a_row_engine(k):
        m = k % 20
        if m < 9:
            return "D"
        if m < 15:
            return "A"
        return "P"

    # lo TT-sub per (b, half, g): 16 total
    def lo_engine(k):
        return "D" if k % 8 < 3 else "P"

    # relu2 per (b, u, n): 208 total; Pool cannot read PSUM
    def relu2_engine(k):
        return "A" if k % 20 < 11 else "D"

    with tile.TileContext(nc) as tc:
        with (
            tc.tile_pool(name="persist", bufs=1) as pp,
            tc.tile_pool(name="apool", bufs=1) as pa,
            tc.tile_pool(name="r8pool", bufs=2) as pr,
            tc.tile_pool(name="work", bufs=3) as pwork,
            tc.tile_pool(name="sigp", bufs=2) as psig,
        ):
            def load(name, shape, dt, src):
                t = pp.tile(shape, dt, tag=name, name=name)
                if len(shape) == 3:
                    nc.sync.dma_start(t[:, :, :], src)
                else:
                    nc.sync.dma_start(t[:, :], src)
                return t

            # DMA order = deadline order.
            basis = load("basis", [128, 1800], fp16, dbasis[:, :])
            xT = [load(f"xT{c}", [DCW[c], B * SC], fp16,
                       dxT[c * 128: c * 128 + DCW[c], :]) for c in range(3)]
            ahat = load("ahat", [128, 4 * SC], fp16, dahat[:, :])
            root = load("root", [128, 900], fp16, droot[:, :])
            bias = load("bias", [128, 3], f32, dbias[:, :])
            w1a = load("w1a", [128, 1536], fp16, dw1a[:, :])
            w1c = load("w1c", [128, 1536], fp16, dw1c[:, :])
            ta0 = load("ta0", [128, 4, S], fp16,
                       dta0[:, :].rearrange("p (m s) -> p m s", m=4))
            tab = load("tab", [128, 4, IPC * BW], fp16,
                       dtab[:, :].rearrange("p (m s) -> p m s", m=4))
            w28t = load("w28t", [128, 2048], fp8, dw28[:, :])
            # per (g, n): [128, 2, 128]
            w28 = [[w28t[:, (g * 4 + n) * 256: (g * 4 + n) * 256 + 256]
                    .rearrange("p (two m) -> p two m", two=2)
                    for n in range(4)] for g in range(2)]
            wp = load("wp", [128, 4], fp16, dwp[:, :])

            hT = [[pp.tile([DCW[ec], SC], fp16, tag=f"hT{b}{ec}", name=f"hT{b}{ec}")
                   for ec in range(3)] for b in range(B)]
            uvT = [[pp.tile([128, SC], fp16, tag=f"uvT{b}{mc}", name=f"uvT{b}{mc}")
                    for mc in range(4)] for b in range(B)]
            vT = [[pp.tile([128, IPC], f32, tag=f"vT{b}{mc}", name=f"vT{b}{mc}")
                   for mc in range(4)] for b in range(B)]
            u2 = [[pp.tile([128, S], fp16, tag=f"u2{b}{mc}", name=f"u2{b}{mc}")
                   for mc in range(4)] for b in range(B)]

            # ---------------- stage A: RGCN h, then u/v ----------
            with tc.tile_pool(name="psA", bufs=2, space="PSUM") as psA:
                t1 = [[[pp.tile([JCW[jc], D], fp16, tag=f"t1_{b}{k}{jc}",
                                name=f"t1_{b}{k}{jc}")
                        for jc in range(2)] for k in range(2)] for b in range(B)]

                def emit_t1(b):
                    for k in range(2):
                        for jc in range(2):
                            t1ps = psA.tile([JCW[jc], D], f32, tag="mps", name="t1ps")
                            for dc in range(3):
                                nc.tensor.matmul(
                                    t1ps[:, :],
                                    xT[dc][:, b * SC + jc * 128: b * SC + jc * 128 + JCW[jc]],
                                    basis[0: DCW[dc], (k * 3 + dc) * D: (k * 3 + dc) * D + D],
                                    start=(dc == 0), stop=(dc == 2),
                                )
                            if b == 0:
                                nc.vector.tensor_copy(t1[b][k][jc][:, :], t1ps[:, :])
                            else:
                                nc.scalar.activation(t1[b][k][jc][:, :], t1ps[:, :], AF.Copy)

                def emit_h(b):
                    for ec in range(3):
                        hps = psA.tile([DCW[ec], SC], f32, tag="hps", name="hps")
                        first = True
                        for k in range(2):
                            for jc in range(2):
                                nc.tensor.matmul(
                                    hps[:, :],
                                    t1[b][k][jc][:, ec * 128: ec * 128 + DCW[ec]],
                                    ahat[0: JCW[jc], (k * 2 + jc) * SC: (k * 2 + jc + 1) * SC],
                                    start=first, stop=False)
                                first = False
                        for dc in range(3):
                            nc.tensor.matmul(
                                hps[:, :],
                                root[0: DCW[dc], dc * D + ec * 128: dc * D + ec * 128 + DCW[ec]],
                                xT[dc][:, b * SC: (b + 1) * SC],
                                start=False, stop=(dc == 2))
                        if b == 0:
                            nc.vector.tensor_scalar(
                                out=hT[b][ec][:, :], in0=hps[:, :],
                                scalar1=bias[0: DCW[ec], ec: ec + 1],
                                scalar2=None, op0=OP.add)
                        else:
                            nc.scalar.activation(hT[b][ec][:, :], hps[:, :], AF.Identity,
                                                 bias=bias[0: DCW[ec], ec: ec + 1])

                def emit_uv(b):
                    for mc in range(4):
                        ups = psA.tile([128, SC], f32, tag="uvps", name="ups")
                        for ec in range(3):
                            nc.tensor.matmul(
                                ups[:, 0:S],
                                w1a[0: DCW[ec], ec * M + mc * 128: ec * M + mc * 128 + 128],
                                hT[b][ec][:, 0:S], start=(ec == 0), stop=False)
                        for ec in range(3):
                            nc.tensor.matmul(
                                ups[:, S:SC],
                                w1c[0: DCW[ec], ec * M + mc * 128: ec * M + mc * 128 + 128],
                                hT[b][ec][:, S:SC], start=(ec == 0), stop=(ec == 2))
                        if b == 0:
                            nc.vector.tensor_copy(uvT[b][mc][:, :], ups[:, :])
                            nc.vector.tensor_copy(vT[b][mc][:, :], ups[:, S:SC])
                        else:
                            nc.scalar.activation(uvT[b][mc][:, :], ups[:, :], AF.Copy)
                            nc.scalar.activation(vT[b][mc][:, :], ups[:, S:SC], AF.Copy)

                emit_t1(0)
                emit_h(0)
                emit_uv(0)
                for b in range(1, B):
                    emit_t1(b)
                    emit_h(b)
                    emit_uv(b)

            # ---------------- rh1 production (a / hi / lo) --------------
            # a4[(b,half)]: [128, 4, rows*S] fp16; r8 hi/lo same shape fp8
            ENG = {"D": nc.vector, "P": nc.gpsimd}
            a_cnt = [0]
            lo_cnt = [0]

            a4 = {}
            r8hi = {}
            r8lo = {}

            def a_chunks(b):
                """Return list of closures; executing all emits a/hi/lo for b."""
                chunks = []

                def c_u2():
                    for mc in range(4):
                        nc.vector.tensor_add(u2[b][mc][:, :], uvT[b][mc][:, 0:S],
                                             ta0[:, mc, :])
                    for half, rows in ((0, HA), (1, HB)):
                        r8hi[(b, half)] = pr.tile([128, 4, rows * S], fp8,
                                                  tag=f"hi{half}", name=f"hi_{b}_{half}")
                        r8lo[(b, half)] = pr.tile([128, 4, rows * S], fp8,
                                                  tag=f"lo{half}", name=f"lo_{b}_{half}")
                chunks.append(c_u2)

                def mk_amc(half, r0, rows, g, sub):
                    mc = 2 * g + sub

                    def c():
                        if sub == 0:
                            a4[(b, half, g)] = pa.tile(
                                [128, 2, rows * S], fp16, tag=f"a{half}{g}",
                                name=f"a_{b}_{half}{g}")
                        at = a4[(b, half, g)]
                        arows = at[:, sub, :].rearrange("p (r c) -> p r c", r=rows)
                        tmid = pwork.tile([128, rows, BW], fp16, tag="tmid",
                                          name=f"tmid{b}{half}{mc}")
                        nc.vector.tensor_add(
                            tmid[:, :, :],
                            uvT[b][mc][:, RW:S].unsqueeze(1)
                            .broadcast_to([128, rows, BW]),
                            tab[:, mc, :].rearrange("p (r c) -> p r c", r=IPC)
                            [:, r0:r0 + rows, :])
                        nc.vector._custom_dve(
                            RELU_TTS, out=arows[:, :, RW:S], in0=tmid[:, :, :],
                            in1=uvT[b][mc][:, S + r0:S + r0 + rows]
                            .unsqueeze(2).broadcast_to([128, rows, BW]),
                            s0=0.0)
                        for r in range(rows):
                            eng = a_row_engine(a_cnt[0]); a_cnt[0] += 1
                            if eng == "A":
                                nc.scalar.activation(
                                    arows[:, r, 0:RW], u2[b][mc][:, 0:RW], AF.Relu,
                                    bias=vT[b][mc][:, r0 + r: r0 + r + 1])
                            else:
                                ENG[eng].tensor_scalar(
                                    out=arows[:, r, 0:RW], in0=u2[b][mc][:, 0:RW],
                                    scalar1=vT[b][mc][:, r0 + r: r0 + r + 1],
                                    scalar2=0.0, op0=OP.add, op1=OP.max)
                    return c

                def mk_hilo(half, g):
                    def c():
                        at = a4[(b, half, g)]
                        hit, lot = r8hi[(b, half)], r8lo[(b, half)]
                        nc.gpsimd.dma_start(hit[:, 2 * g:2 * g + 2, :],
                                            at[:, :, :])
                        eng = lo_engine(lo_cnt[0]); lo_cnt[0] += 1
                        ENG[eng].tensor_sub(lot[:, 2 * g:2 * g + 2, :],
                                            at[:, :, :],
                                            hit[:, 2 * g:2 * g + 2, :])
                    return c

                for half, (r0, rows) in enumerate(((0, HA), (HA, HB))):
                    for g in range(2):
                        for sub in range(2):
                            chunks.append(mk_amc(half, r0, rows, g, sub))
                        chunks.append(mk_hilo(half, g))
                return chunks

            # ---------------- stage B: GEMM2 / relu2 / GEMM3 ------------
            r2_cnt = [0]

            def emit_g3(prev):
                p_u, p_nch, p_pout, p_rh2 = prev
                for pc in range(p_nch):
                    col = p_u * 4 + pc
                    for mc in range(4):
                        nc.tensor.matmul(
                            p_pout[0:100, col: col + 1],
                            p_rh2[mc][:, pc * 100: pc * 100 + 100],
                            wp[:, mc: mc + 1],
                            start=(mc == 0), stop=(mc == 3))

            with (
                tc.tile_pool(name="ps2", bufs=5, space="PSUM") as ps2,
                tc.tile_pool(name="pp3", bufs=2, space="PSUM") as pp3,
            ):
                # b0's rh1 is emitted fully up front (overlaps stage A of
                # b1..b3 on PE); b+1's chunks interleave with stage-B(b).
                for ch in a_chunks(0):
                    ch()

                prev = None
                for b in range(B):
                    nxt = a_chunks(b + 1) if b + 1 < B else []
                    pout = pp3.tile([128, NCOL], f32, tag="pout", name="pout")
                    sig = psig.tile([128, NCOL], f32, tag="sigb", name="sigb")
                    for u in range(NU):
                        half = 0 if u < 6 else 1
                        ubase = u * 400 if half == 0 else (u - 6) * 400
                        nil = 2 if u < NU - 1 else 1
                        ncols = nil * S
                        hit, lot = r8hi[(b, half)], r8lo[(b, half)]
                        rh2 = [pwork.tile([128, 400], fp16, tag=f"rh2_{n}",
                                          name=f"rh2_{n}") for n in range(4)]
                        for n in range(4):
                            ops = ps2.tile([128, 400], f32, tag="ops", name="ops")
                            for g in range(2):
                                for t8 in (hit, lot):
                                    nc.tensor.matmul(
                                        ops[:, :ncols],
                                        w28[g][n][:, :, :],
                                        t8[:, 2 * g:2 * g + 2, ubase:ubase + ncols],
                                        start=(g == 0 and t8 is hit),
                                        stop=(g == 1 and t8 is lot),
                                        perf_mode=PM.DoubleRow)
                            eng = relu2_engine(r2_cnt[0]); r2_cnt[0] += 1
                            if eng == "A":
                                nc.scalar.activation(rh2[n][:, :ncols], ops[:, :ncols],
                                                     AF.Relu)
                            else:
                                ENG[eng].tensor_scalar(
                                    out=rh2[n][:, :ncols], in0=ops[:, :ncols],
                                    scalar1=0.0, scalar2=None, op0=OP.max)
                        if prev is not None:
                            emit_g3(prev)
                        prev = (u, 4 if nil == 2 else 2, pout, rh2)
                        # interleave next batch's rh1 work: ~14 chunks over
                        # 13 units
                        if nxt:
                            nxt.pop(0)()
                            if u == NU - 1:
                                while nxt:
                                    nxt.pop(0)()
                        if u == 7:
                            nc.scalar.activation(sig[0:100, 0:24], pout[0:100, 0:24],
                                                 AF.Sigmoid)
                            nc.sync.dma_start(dout[b * 100: b * 100 + 100, 0:24],
                                              sig[0:100, 0:24])
                    nc.scalar.activation(sig[0:100, 24:48], pout[0:100, 24:48],
                                         AF.Sigmoid)
                    nc.sync.dma_start(dout[b * 100: b * 100 + 100, 24:48],
                                      sig[0:100, 24:48])
                    emit_g3(prev)
                    prev = None
                    nc.scalar.activation(sig[0:100, 48:NCOL], pout[0:100, 48:NCOL],
                                         AF.Sigmoid)
                    nc.sync.dma_start(dout[b * 100: b * 100 + 100, 48:NCOL],
                                      sig[0:100, 48:NCOL])

    nc.compile()
    return nc


def _host_prep(x, pe_k, pe_v, comp, basis, root, rgcn_bias, W1, W2, Wp):
    import ml_dtypes

    h16 = np.float16
    f8 = ml_dtypes.float8_e4m3

    ra = _rel_adj(S) % NREL
    onehot = (ra[None, :, :] == np.arange(NREL)[:, None, None]).astype(np.float64)
    deg = onehot.sum(1)
    inv = np.where(deg > 0, 1.0 / np.maximum(deg, 1.0), 0.0)
    anorm = onehot * inv[:, None, :]
    ahat_full = np.einsum("rk,rij->kij", np.asarray(comp, np.float64), anorm)
    ahat_full = ahat_full.astype(np.float32)  # [2, S, S]  (i=src, j=tgt)
    pos = np.clip(np.arange(S)[:, None] - np.arange(S)[None, :] + 1, 0, MAXL)

    x = np.asarray(x, np.float32)
    W1 = np.asarray(W1, np.float32)
    W1a, W1b = W1[:D], W1[D: D + P]
    W1c, W1d = W1[D + P: 2 * D + P], W1[2 * D + P:]
    ttab = (np.asarray(pe_k, np.float64) @ W1b.astype(np.float64)
            + np.asarray(pe_v, np.float64) @ W1d.astype(np.float64)).astype(np.float32)
    ttab = ttab.astype(h16).astype(np.float32)  # [11, 512] as the device sees it

    W2 = np.asarray(W2, np.float32)
    w28 = np.zeros((128, 2048), f8)
    for g in range(2):
        for n in range(4):
            for a in range(2):
                blk = W2[g * 256 + a * 128: g * 256 + a * 128 + 128,
                         n * 128: n * 128 + 128]
                w28[:, (g * 4 + n) * 256 + a * 128: (g * 4 + n) * 256 + a * 128 + 128] = \
                    blk.astype(f8)

    com = {
        "basis": np.concatenate(
            [_pack_k(np.asarray(basis[k], np.float32)) for k in range(2)], axis=1
        ).astype(h16),
        "root": _pack_k(np.asarray(root, np.float32)).astype(h16),
        "w1a": _pack_k(W1a).astype(h16),
        "w1c": _pack_k(W1c).astype(h16),
        "w28": w28,
        "wp": np.ascontiguousarray(np.asarray(Wp, np.float32)[:, 0]
                                   .reshape(4, 128).T).astype(h16),
    }
    bias_p = np.zeros((128, 3), np.float32)
    rb = np.asarray(rgcn_bias, np.float32)
    for c in range(3):
        r = min(128, D - c * 128)
        bias_p[:r, c] = rb[c * 128: c * 128 + r]
    com["bias"] = bias_p

    xt_all = x.transpose(2, 0, 1)  # [D, B, S]
    per_core = []
    perms = []
    for c in range(NCORES):
        i0 = c * IPC
        perm = (i0 + 26 + np.arange(S)) % S  # j' -> j
        perms.append(perm)
        m = dict(com)
        # xT: j columns in perm order, then the i-slice
        xtc = np.empty((D, B * SC), np.float32)
        for b in range(B):
            xtc[:, b * SC: b * SC + S] = xt_all[:, b, perm]
            xtc[:, b * SC + S: (b + 1) * SC] = xt_all[:, b, i0: i0 + IPC]
        m["xT"] = xtc.astype(h16)
        # ahat: rows = source in perm order, cols = target (perm ++ i-slice)
        ah = np.zeros((128, 4 * SC), np.float32)
        for k in range(2):
            ap = ahat_full[k][perm][:, :]  # [S(src, perm), S(tgt)]
            for jc in range(2):
                r = JW = 128 if jc == 0 else 72
                base = (k * 2 + jc) * SC
                ah[:r, base: base + S] = ap[jc * 128: jc * 128 + r][:, perm]
                ah[:r, base + S: base + SC] = ap[jc * 128: jc * 128 + r][:, i0: i0 + IPC]
        m["ahat"] = ah.astype(h16)
        # TA0: row-independent T value for rect cols (0 for band cols)
        rows = np.arange(i0, i0 + IPC)
        ta0 = np.zeros((128, 4 * S), np.float32)
        tabv = np.zeros((128, 4 * IPC * BW), np.float32)
        pv = pos[np.ix_(rows, perm)]  # [IPC, S] pos values in j' order
        assert (pv[:, :RW] == pv[0:1, :RW]).all(), "rect cols not row-const"
        for mc in range(4):
            tcols = ttab[:, mc * 128: (mc + 1) * 128]  # [11, 128]
            ta0[:, mc * S: mc * S + RW] = tcols[pv[0, :RW]].T
            tb = tcols[pv[:, RW:]]  # [IPC, BW, 128]
            tabv[:, mc * IPC * BW: (mc + 1) * IPC * BW] = \
                tb.transpose(2, 0, 1).reshape(128, IPC * BW)
        m["ta0"] = ta0.astype(h16)
        m["tab"] = tabv.astype(h16)
        per_core.append(m)
    return per_core, perms


def kernel(x, mask, pe_k, pe_v, comp, basis, root, rgcn_bias, W1, W2, Wp,
           _want_results=False, _trace=False):
    from concourse.bass_utils import run_bass_kernel_spmd

    if "nc" not in _prog_cache:
        _prog_cache["nc"] = _build_program()
    nc = _prog_cache["nc"]

    in_maps, perms = _host_prep(x, pe_k, pe_v, comp, basis, root, rgcn_bias,
                                W1, W2, Wp)
    res = run_bass_kernel_spmd(nc, in_maps, core_ids=list(range(NCORES)),
                               trace=_trace)

    out = np.zeros((B, S, S), np.float32)
    for c in range(NCORES):
        i0 = c * IPC
        arr = np.asarray(res.results[c]["out"], np.float32).reshape(B, 100, NCOL)
        # pair linear order: col*100 + q -> (i_rel, j') with
        # i_rel = (col*100+q)//200, j' = (col*100+q)%200
        blk = arr.transpose(0, 2, 1).reshape(B, IPC, S)  # [B, i_rel, j']
        out[:, i0: i0 + IPC, perms[c]] = blk
    out *= np.asarray(mask, np.float32)
    if _want_results:
        return out, res
    return out


# revision 10
# speedup vs baseline: 1.3912x; 1.3912x over previous
"""Trainium2 Bass kernel for nn_CausePredictor (RGCN + pairwise MLP).

Sharding: data-parallel over the pairwise row index i: 8 cores x 25 rows,
replicated over B=4.  All per-core differences are encoded as input DATA,
so one SPMD program serves all cores.

Math (matching reference.py):
  h   = sum_k Ahat_k.T @ (x[b] @ basis_k) + x[b] @ root + bias
  u   = h @ W1a   (j term),  v = h @ W1c  (i term)
  h1[b,i,j,:] = u[b,j] + v[b,i] + T[pos(i,j)]
  out = sigmoid(Wp . relu(relu(h1) @ W2)) * mask

v2 performance structure (TimelineSim cost model):
  - GEMM2 runs FULLY in fp8 DoubleRow (8 cyc/col vs v1's 12): rh1 is
    split into an fp8 hi stream + an fp8 lo residual stream (hi+lo is
    ~fp16-exact), W2 is single fp8.  Modeled max-rel-err 1.51e-2.
  - All carriers are fp16 (not bf16) for mantissa headroom.
  - The T[pos] table is NOT pair-expanded on host.  A per-core column
    permutation of j (baked into xT/ahat data, inverted on unshard)
    puts the 35-wide diagonal T-band at fixed columns [165,200) for
    every core; the remaining 165 "rect" columns have a row-independent
    T value folded into u'' = u + TA0 (one tiny TT per (b,mc)).
  - a = relu(u'' + v) in fp16: per-row stock tensor_scalar (DVE 2x /
    Act activation / Pool TS); band: TT + one custom DVE op
    relu(Src0+Src1) with v broadcast.
  - hi = fp8 cast of a via gpsimd CASTING DMA (free engine-wise).
  - lo = a - hi via tensor_sub on DVE/Pool.
  - relu2 (PSUM->SBUF) mostly on Act; GEMM3 with stationary rh2 and
    moving wp (1-col outputs, ~free) as in v1.
"""

import sys

sys.path.insert(0, "/opt/trn_rl_repo")

import numpy as np

B, S, D, M, P = 4, 200, 300, 512, 100
NREL, MAXL = 9, 10
NCORES = 8
IPC = S // NCORES          # 25 rows of i per core
SC = S + IPC               # 225: u columns ++ the core's i-slice
RW = 165                   # rect columns [0,165)
BW = S - RW                # 35 band columns [165,200)
FPC = IPC * S              # 5000 pairs per (b, core)
NCOL = 50                  # output columns per b (100 pairs each)
NU = 13                    # 12x 2-row units + 1x 1-row unit per b
HA, HB = 12, 13            # row halves: units 0..5 / 6..12

_prog_cache = {}


def _rel_adj(s):
    ra = np.arange(s)[None, :] - np.arange(s)[:, None]
    for i in range(s):
        ra[i, i + 1:] = 1
        num = 1
        for o in range(i - 1, -1, -2):
            ra[i, o] = -num
            if o - 1 >= 0:
                ra[i, o - 1] = -num
            num += 1
        ra[i, :i] = np.maximum(ra[i, :i], -8)
    return ra


def _pack_k(w):
    """[K, N] -> [128, ceil(K/128)*N], K chunked onto partitions."""
    k, n = w.shape
    nch = (k + 127) // 128
    out = np.zeros((128, nch * n), np.float32)
    for c in range(nch):
        r = min(128, k - c * 128)
        out[:r, c * n: c * n + n] = w[c * 128: c * 128 + r]
    return out


# ---------------- custom DVE op ------------------------------------------
def _register_custom_ops():
    from concourse.dve_spec import Spec, Src0, Src1, C0, relu, lower
    from concourse import dve_ops as dvo
    from concourse.dve_uop import DveOpSpec

    if "RELU_TTS_ANT" in dvo.CUSTOM_DVE_SPECS:
        return next(o for o in dvo.OPS if o.name == "RELU_TTS_ANT")
    spec = Spec(
        body=relu(Src0 + Src1 + C0),
        reference=lambda in0, in1, s0, s1, imm2: np.maximum(
            in0.astype(np.float32) + in1 + s0, 0.0),
    )
    shas = {}
    for ver in ("v3", "v4"):
        s = DveOpSpec(name="RELU_TTS_ANT", opcode=0, uops=lower(spec, ver=ver),
                      rd1_en=True)
        shas[ver] = s.sha(ver)
    op = dvo.DveOp("RELU_TTS_ANT", spec, subdim=False, uops_sha=shas)
    dvo.OPS.append(op)
    dvo.CUSTOM_DVE_SPECS[op.name] = spec
    dvo._SUB_OPCODE_FOR_NAME[op.name] = dvo._CUSTOM_DVE_ROW_BASE + len(dvo.OPS) - 1
    return op


def _build_program():
    import ml_dtypes  # noqa: F401
    import concourse.tile as tile
    from concourse import bacc, mybir

    RELU_TTS = _register_custom_ops()

    f32 = mybir.dt.float32
    fp16 = mybir.dt.float16
    fp8 = mybir.dt.float8e4
    AF = mybir.ActivationFunctionType
    OP = mybir.AluOpType
    PM = mybir.MatmulPerfMode

    nc = bacc.Bacc()

    dxT = nc.declare_dram_parameter("xT", [D, B * SC], fp16, isOutput=False)
    dahat = nc.declare_dram_parameter("ahat", [128, 4 * SC], fp16, isOutput=False)
    dbasis = nc.declare_dram_parameter("basis", [128, 1800], fp16, isOutput=False)
    droot = nc.declare_dram_parameter("root", [128, 900], fp16, isOutput=False)
    dbias = nc.declare_dram_parameter("bias", [128, 3], f32, isOutput=False)
    dw1a = nc.declare_dram_parameter("w1a", [128, 1536], fp16, isOutput=False)
    dw1c = nc.declare_dram_parameter("w1c", [128, 1536], fp16, isOutput=False)
    dw28 = nc.declare_dram_parameter("w28", [128, 4096], fp8, isOutput=False)
    dwp = nc.declare_dram_parameter("wp", [128, 4], fp16, isOutput=False)
    dta0 = nc.declare_dram_parameter("ta0", [128, 4 * S], fp16, isOutput=False)
    dtab = nc.declare_dram_parameter("tab", [128, 4 * IPC * BW], fp16, isOutput=False)
    dout = nc.declare_dram_parameter("out", [B * 100, NCOL], f32, isOutput=True)

    DCW = [128, 128, 44]   # D=300 chunks
    JCW = [128, 72]        # S=200 chunks

    # ---- engine assignment policies (tuning knobs) ----
    # a-row engine by linear index (b, mc, half, r)
    def a_row_engine(k):
        m = k % 40
        if m < 13:
            return "D"
        if m < 14:
            return "A"
        return "P"

    # lo TT-sub per (b, half): 8 total (group 1 only)
    def lo_engine(k):
        return "D"

    # relu2 per (b, u, np2): 104 wide instrs; Pool cannot read PSUM
    def relu2_engine(k):
        return "A" if k % 10 < 9 else "D"

    with tile.TileContext(nc) as tc:
        with (
            tc.tile_pool(name="persist", bufs=1) as pp,
            tc.tile_pool(name="apool", bufs=1) as pa,
            tc.tile_pool(name="r8pool", bufs=2) as pr,
            tc.tile_pool(name="work", bufs=3) as pwork,
            tc.tile_pool(name="sigp", bufs=2) as psig,
        ):
            def load(name, shape, dt, src):
                t = pp.tile(shape, dt, tag=name, name=name)
                if len(shape) == 3:
                    nc.sync.dma_start(t[:, :, :], src)
                else:
                    nc.sync.dma_start(t[:, :], src)
                return t

            # DMA order = deadline order.
            basis = load("basis", [128, 1800], fp16, dbasis[:, :])
            xT = [load(f"xT{c}", [DCW[c], B * SC], fp16,
                       dxT[c * 128: c * 128 + DCW[c], :]) for c in range(3)]
            ahat = load("ahat", [128, 4 * SC], fp16, dahat[:, :])
            root = load("root", [128, 900], fp16, droot[:, :])
            bias = load("bias", [128, 3], f32, dbias[:, :])
            w1a = load("w1a", [128, 1536], fp16, dw1a[:, :])
            w1c = load("w1c", [128, 1536], fp16, dw1c[:, :])
            ta0 = load("ta0", [128, 4, S], fp16,
                       dta0[:, :].rearrange("p (m s) -> p m s", m=4))
            tab = load("tab", [128, 4, IPC * BW], fp16,
                       dtab[:, :].rearrange("p (m s) -> p m s", m=4))
            w28t = load("w28t", [128, 4096], fp8, dw28[:, :])
            # per (g, n, s) with s=0 hi, s=1 lo: [128, 2, 128]
            w28 = [[[w28t[:, ((g * 4 + n) * 2 + s) * 256: ((g * 4 + n) * 2 + s) * 256 + 256]
                     .rearrange("p (two m) -> p two m", two=2)
                     for s in range(2)] for n in range(4)] for g in range(2)]
            wp = load("wp", [128, 4], fp16, dwp[:, :])

            hT = [[pp.tile([DCW[ec], SC], fp16, tag=f"hT{b}{ec}", name=f"hT{b}{ec}")
                   for ec in range(3)] for b in range(B)]
            uvT = [[pp.tile([128, SC], fp16, tag=f"uvT{b}{mc}", name=f"uvT{b}{mc}")
                    for mc in range(4)] for b in range(B)]
            vT = [[pp.tile([128, IPC], f32, tag=f"vT{b}{mc}", name=f"vT{b}{mc}")
                   for mc in range(4)] for b in range(B)]
            u2 = [[pp.tile([128, S], fp16, tag=f"u2{b}{mc}", name=f"u2{b}{mc}")
                   for mc in range(4)] for b in range(B)]

            # ---------------- stage A: RGCN h, then u/v ----------
            with tc.tile_pool(name="psA", bufs=2, space="PSUM") as psA:
                t1 = [[[pp.tile([JCW[jc], D], fp16, tag=f"t1_{b}{k}{jc}",
                                name=f"t1_{b}{k}{jc}")
                        for jc in range(2)] for k in range(2)] for b in range(B)]

                def emit_t1(b):
                    for k in range(2):
                        for jc in range(2):
                            t1ps = psA.tile([JCW[jc], D], f32, tag="mps", name="t1ps")
                            for dc in range(3):
                                nc.tensor.matmul(
                                    t1ps[:, :],
                                    xT[dc][:, b * SC + jc * 128: b * SC + jc * 128 + JCW[jc]],
                                    basis[0: DCW[dc], (k * 3 + dc) * D: (k * 3 + dc) * D + D],
                                    start=(dc == 0), stop=(dc == 2),
                                )
                            if b == 0:
                                nc.vector.tensor_copy(t1[b][k][jc][:, :], t1ps[:, :])
                            else:
                                nc.scalar.activation(t1[b][k][jc][:, :], t1ps[:, :], AF.Copy)

                def emit_h(b):
                    for ec in range(3):
                        hps = psA.tile([DCW[ec], SC], f32, tag="hps", name="hps")
                        first = True
                        for k in range(2):
                            for jc in range(2):
                                nc.tensor.matmul(
                                    hps[:, :],
                                    t1[b][k][jc][:, ec * 128: ec * 128 + DCW[ec]],
                                    ahat[0: JCW[jc], (k * 2 + jc) * SC: (k * 2 + jc + 1) * SC],
                                    start=first, stop=False)
                                first = False
                        for dc in range(3):
                            nc.tensor.matmul(
                                hps[:, :],
                                root[0: DCW[dc], dc * D + ec * 128: dc * D + ec * 128 + DCW[ec]],
                                xT[dc][:, b * SC: (b + 1) * SC],
                                start=False, stop=(dc == 2))
                        if b == 0:
                            nc.vector.tensor_scalar(
                                out=hT[b][ec][:, :], in0=hps[:, :],
                                scalar1=bias[0: DCW[ec], ec: ec + 1],
                                scalar2=None, op0=OP.add)
                        else:
                            nc.scalar.activation(hT[b][ec][:, :], hps[:, :], AF.Identity,
                                                 bias=bias[0: DCW[ec], ec: ec + 1])

                def emit_uv(b):
                    for mc in range(4):
                        ups = psA.tile([128, SC], f32, tag="uvps", name="ups")
                        for ec in range(3):
                            nc.tensor.matmul(
                                ups[:, 0:S],
                                w1a[0: DCW[ec], ec * M + mc * 128: ec * M + mc * 128 + 128],
                                hT[b][ec][:, 0:S], start=(ec == 0), stop=False)
                        for ec in range(3):
                            nc.tensor.matmul(
                                ups[:, S:SC],
                                w1c[0: DCW[ec], ec * M + mc * 128: ec * M + mc * 128 + 128],
                                hT[b][ec][:, S:SC], start=(ec == 0), stop=(ec == 2))
                        if b == 0:
                            nc.vector.tensor_copy(uvT[b][mc][:, :], ups[:, :])
                            nc.vector.tensor_copy(vT[b][mc][:, :], ups[:, S:SC])
                        else:
                            nc.scalar.activation(uvT[b][mc][:, :], ups[:, :], AF.Copy)
                            nc.scalar.activation(vT[b][mc][:, :], ups[:, S:SC], AF.Copy)

                emit_t1(0)
                emit_h(0)
                emit_uv(0)
                for b in range(1, B):
                    emit_t1(b)
                    emit_h(b)
                    emit_uv(b)

            # ---------------- rh1 production (a / hi / lo) --------------
            # a4[(b,half)]: [128, 4, rows*S] fp16; r8 hi/lo same shape fp8
            ENG = {"D": nc.vector, "P": nc.gpsimd}
            a_cnt = [0]
            lo_cnt = [0]

            a4 = {}
            r8hi = {}
            r8lo = {}

            def a_chunks(b):
                """Return list of closures; executing all emits a/hi/lo for b."""
                chunks = []

                def c_u2():
                    for mc in range(4):
                        nc.vector.tensor_add(u2[b][mc][:, :], uvT[b][mc][:, 0:S],
                                             ta0[:, mc, :])
                    for half, rows in ((0, HA), (1, HB)):
                        r8hi[(b, half)] = pr.tile([128, 4, rows * S], fp8,
                                                  tag=f"hi{half}", name=f"hi_{b}_{half}")
                        r8lo[(b, half)] = pr.tile([128, 2, rows * S], fp8,
                                                  tag=f"lo{half}", name=f"lo_{b}_{half}")
                chunks.append(c_u2)

                def mk_amc(half, r0, rows, g, sub):
                    mc = 2 * g + sub

                    def c():
                        if sub == 0:
                            a4[(b, half, g)] = pa.tile(
                                [128, 2, rows * S], fp16, tag=f"a{half}{g}",
                                name=f"a_{b}_{half}{g}")
                        at = a4[(b, half, g)]
                        arows = at[:, sub, :].rearrange("p (r c) -> p r c", r=rows)
                        tmid = pwork.tile([128, rows, BW], fp16, tag="tmid",
                                          name=f"tmid{b}{half}{mc}")
                        nc.vector.tensor_add(
                            tmid[:, :, :],
                            uvT[b][mc][:, RW:S].unsqueeze(1)
                            .broadcast_to([128, rows, BW]),
                            tab[:, mc, :].rearrange("p (r c) -> p r c", r=IPC)
                            [:, r0:r0 + rows, :])
                        nc.vector._custom_dve(
                            RELU_TTS, out=arows[:, :, RW:S], in0=tmid[:, :, :],
                            in1=uvT[b][mc][:, S + r0:S + r0 + rows]
                            .unsqueeze(2).broadcast_to([128, rows, BW]),
                            s0=0.0)
                        for r in range(rows):
                            eng = a_row_engine(a_cnt[0]); a_cnt[0] += 1
                            if eng == "A":
                                nc.scalar.activation(
                                    arows[:, r, 0:RW], u2[b][mc][:, 0:RW], AF.Relu,
                                    bias=vT[b][mc][:, r0 + r: r0 + r + 1])
                            else:
                                ENG[eng].tensor_scalar(
                                    out=arows[:, r, 0:RW], in0=u2[b][mc][:, 0:RW],
                                    scalar1=vT[b][mc][:, r0 + r: r0 + r + 1],
                                    scalar2=0.0, op0=OP.add, op1=OP.max)
                    return c

                def mk_hilo(half, g):
                    def c():
                        at = a4[(b, half, g)]
                        hit = r8hi[(b, half)]
                        nc.gpsimd.dma_start(hit[:, 2 * g:2 * g + 2, :],
                                            at[:, :, :])
                        if g == 1:
                            lot = r8lo[(b, half)]
                            eng = lo_engine(lo_cnt[0]); lo_cnt[0] += 1
                            ENG[eng].tensor_sub(lot[:, :, :],
                                                at[:, :, :],
                                                hit[:, 2:4, :])
                    return c

                for half, (r0, rows) in enumerate(((0, HA), (HA, HB))):
                    for g in range(2):
                        for sub in range(2):
                            chunks.append(mk_amc(half, r0, rows, g, sub))
                        chunks.append(mk_hilo(half, g))
                return chunks

            # ---------------- stage B: GEMM2 / relu2 / GEMM3 ------------
            r2_cnt = [0]

            def emit_g3(prev):
                p_u, p_nch, p_pout, p_rh2 = prev
                for pc in range(p_nch):
                    col = p_u * 4 + pc
                    for mc in range(4):
                        nc.tensor.matmul(
                            p_pout[0:100, col: col + 1],
                            p_rh2[mc // 2][:, mc % 2, pc * 100: pc * 100 + 100],
                            wp[:, mc: mc + 1],
                            start=(mc == 0), stop=(mc == 3))

            with (
                tc.tile_pool(name="ps2", bufs=3, space="PSUM") as ps2,
                tc.tile_pool(name="pp3", bufs=2, space="PSUM") as pp3,
            ):
                # b0's rh1 is emitted fully up front (overlaps stage A of
                # b1..b3 on PE); b+1's chunks interleave with stage-B(b).
                for ch in a_chunks(0):
                    ch()

                prev = None
                for b in range(B):
                    nxt = a_chunks(b + 1) if b + 1 < B else []
                    pout = pp3.tile([128, NCOL], f32, tag="pout", name="pout")
                    sig = psig.tile([128, NCOL], f32, tag="sigb", name="sigb")
                    for u in range(NU):
                        half = 0 if u < 6 else 1
                        ubase = u * 400 if half == 0 else (u - 6) * 400
                        nil = 2 if u < NU - 1 else 1
                        ncols = nil * S
                        hit, lot = r8hi[(b, half)], r8lo[(b, half)]
                        rh2 = []
                        for np2 in range(2):
                            opsw = ps2.tile([128, 2, 512], f32, tag="ops",
                                            name=f"ops{np2}")
                            for sub in range(2):
                                n = np2 * 2 + sub
                                o = opsw[:, sub, :ncols]
                                # g0: hi@Whi0 + hi@Wlo0; g1: hi@Whi1 + hi@Wlo1
                                # + lo@Whi1
                                nc.tensor.matmul(
                                    o, w28[0][n][0][:, :, :],
                                    hit[:, 0:2, ubase:ubase + ncols],
                                    start=True, stop=False, perf_mode=PM.DoubleRow)
                                nc.tensor.matmul(
                                    o, w28[0][n][1][:, :, :],
                                    hit[:, 0:2, ubase:ubase + ncols],
                                    start=False, stop=False, perf_mode=PM.DoubleRow)
                                nc.tensor.matmul(
                                    o, w28[1][n][0][:, :, :],
                                    hit[:, 2:4, ubase:ubase + ncols],
                                    start=False, stop=False, perf_mode=PM.DoubleRow)
                                nc.tensor.matmul(
                                    o, w28[1][n][1][:, :, :],
                                    hit[:, 2:4, ubase:ubase + ncols],
                                    start=False, stop=False, perf_mode=PM.DoubleRow)
                                nc.tensor.matmul(
                                    o, w28[1][n][0][:, :, :],
                                    lot[:, 0:2, ubase:ubase + ncols],
                                    start=False, stop=True, perf_mode=PM.DoubleRow)
                            rh2w = pwork.tile([128, 2, 400], fp16, tag=f"rh2w{np2}",
                                              name=f"rh2w{np2}")
                            rh2.append(rh2w)
                            eng = relu2_engine(r2_cnt[0]); r2_cnt[0] += 1
                            if eng == "A":
                                nc.scalar.activation(rh2w[:, :, :ncols],
                                                     opsw[:, :, :ncols],
                                                     AF.Relu, scale=1.0 / 16.0)
                            else:
                                nc.vector.tensor_scalar(
                                    out=rh2w[:, :, :ncols], in0=opsw[:, :, :ncols],
                                    scalar1=1.0 / 16.0, scalar2=0.0,
                                    op0=OP.mult, op1=OP.max)
                        if prev is not None:
                            emit_g3(prev)
                        prev = (u, 4 if nil == 2 else 2, pout, rh2)
                        # interleave next batch's rh1 work: ~14 chunks over
                        # 13 units
                        if nxt:
                            nxt.pop(0)()
                            if u == NU - 1:
                                while nxt:
                                    nxt.pop(0)()
                        if u == 7:
                            nc.scalar.activation(sig[0:100, 0:24], pout[0:100, 0:24],
                                                 AF.Sigmoid)
                            nc.sync.dma_start(dout[b * 100: b * 100 + 100, 0:24],
                                              sig[0:100, 0:24])
                    nc.scalar.activation(sig[0:100, 24:48], pout[0:100, 24:48],
                                         AF.Sigmoid)
                    nc.sync.dma_start(dout[b * 100: b * 100 + 100, 24:48],
                                      sig[0:100, 24:48])
                    emit_g3(prev)
                    prev = None
                    nc.scalar.activation(sig[0:100, 48:NCOL], pout[0:100, 48:NCOL],
                                         AF.Sigmoid)
                    nc.sync.dma_start(dout[b * 100: b * 100 + 100, 48:NCOL],
                                      sig[0:100, 48:NCOL])

    nc.compile()
    return nc


def _host_prep(x, pe_k, pe_v, comp, basis, root, rgcn_bias, W1, W2, Wp):
    import ml_dtypes

    h16 = np.float16
    f8 = ml_dtypes.float8_e4m3

    ra = _rel_adj(S) % NREL
    onehot = (ra[None, :, :] == np.arange(NREL)[:, None, None]).astype(np.float64)
    deg = onehot.sum(1)
    inv = np.where(deg > 0, 1.0 / np.maximum(deg, 1.0), 0.0)
    anorm = onehot * inv[:, None, :]
    ahat_full = np.einsum("rk,rij->kij", np.asarray(comp, np.float64), anorm)
    ahat_full = ahat_full.astype(np.float32)  # [2, S, S]  (i=src, j=tgt)
    pos = np.clip(np.arange(S)[:, None] - np.arange(S)[None, :] + 1, 0, MAXL)

    x = np.asarray(x, np.float32)
    W1 = np.asarray(W1, np.float32)
    W1a, W1b = W1[:D], W1[D: D + P]
    W1c, W1d = W1[D + P: 2 * D + P], W1[2 * D + P:]
    ttab = (np.asarray(pe_k, np.float64) @ W1b.astype(np.float64)
            + np.asarray(pe_v, np.float64) @ W1d.astype(np.float64)).astype(np.float32)
    ttab = ttab.astype(h16).astype(np.float32)  # [11, 512] as the device sees it

    W2 = np.asarray(W2, np.float32) * 16.0  # PSUM rescaled 1/16 at relu2
    w28 = np.zeros((128, 4096), f8)
    for g in range(2):
        for n in range(4):
            for a in range(2):
                blk = W2[g * 256 + a * 128: g * 256 + a * 128 + 128,
                         n * 128: n * 128 + 128]
                hi = blk.astype(f8)
                lo = (blk - hi.astype(np.float32)).astype(f8)
                base = (g * 4 + n) * 512 + a * 128
                w28[:, base: base + 128] = hi
                w28[:, base + 256: base + 256 + 128] = lo

    com = {
        "basis": np.concatenate(
            [_pack_k(np.asarray(basis[k], np.float32)) for k in range(2)], axis=1
        ).astype(h16),
        "root": _pack_k(np.asarray(root, np.float32)).astype(h16),
        "w1a": _pack_k(W1a).astype(h16),
        "w1c": _pack_k(W1c).astype(h16),
        "w28": w28,
        "wp": np.ascontiguousarray(np.asarray(Wp, np.float32)[:, 0]
                                   .reshape(4, 128).T).astype(h16),
    }
    bias_p = np.zeros((128, 3), np.float32)
    rb = np.asarray(rgcn_bias, np.float32)
    for c in range(3):
        r = min(128, D - c * 128)
        bias_p[:r, c] = rb[c * 128: c * 128 + r]
    com["bias"] = bias_p

    xt_all = x.transpose(2, 0, 1)  # [D, B, S]
    per_core = []
    perms = []
    for c in range(NCORES):
        i0 = c * IPC
        perm = (i0 + 26 + np.arange(S)) % S  # j' -> j
        perms.append(perm)
        m = dict(com)
        # xT: j columns in perm order, then the i-slice
        xtc = np.empty((D, B * SC), np.float32)
        for b in range(B):
            xtc[:, b * SC: b * SC + S] = xt_all[:, b, perm]
            xtc[:, b * SC + S: (b + 1) * SC] = xt_all[:, b, i0: i0 + IPC]
        m["xT"] = xtc.astype(h16)
        # ahat: rows = source in perm order, cols = target (perm ++ i-slice)
        ah = np.zeros((128, 4 * SC), np.float32)
        for k in range(2):
            ap = ahat_full[k][perm][:, :]  # [S(src, perm), S(tgt)]
            for jc in range(2):
                r = JW = 128 if jc == 0 else 72
                base = (k * 2 + jc) * SC
                ah[:r, base: base + S] = ap[jc * 128: jc * 128 + r][:, perm]
                ah[:r, base + S: base + SC] = ap[jc * 128: jc * 128 + r][:, i0: i0 + IPC]
        m["ahat"] = ah.astype(h16)
        # TA0: row-independent T value for rect cols (0 for band cols)
        rows = np.arange(i0, i0 + IPC)
        ta0 = np.zeros((128, 4 * S), np.float32)
        tabv = np.zeros((128, 4 * IPC * BW), np.float32)
        pv = pos[np.ix_(rows, perm)]  # [IPC, S] pos values in j' order
        assert (pv[:, :RW] == pv[0:1, :RW]).all(), "rect cols not row-const"
        for mc in range(4):
            tcols = ttab[:, mc * 128: (mc + 1) * 128]  # [11, 128]
            ta0[:, mc * S: mc * S + RW] = tcols[pv[0, :RW]].T
            tb = tcols[pv[:, RW:]]  # [IPC, BW, 128]
            tabv[:, mc * IPC * BW: (mc + 1) * IPC * BW] = \
                tb.transpose(2, 0, 1).reshape(128, IPC * BW)
        m["ta0"] = ta0.astype(h16)
        m["tab"] = tabv.astype(h16)
        per_core.append(m)
    return per_core, perms


def kernel(x, mask, pe_k, pe_v, comp, basis, root, rgcn_bias, W1, W2, Wp,
           _want_results=False, _trace=False):
    from concourse.bass_utils import run_bass_kernel_spmd

    if "nc" not in _prog_cache:
        _prog_cache["nc"] = _build_program()
    nc = _prog_cache["nc"]

    in_maps, perms = _host_prep(x, pe_k, pe_v, comp, basis, root, rgcn_bias,
                                W1, W2, Wp)
    res = run_bass_kernel_spmd(nc, in_maps, core_ids=list(range(NCORES)),
                               trace=_trace)

    out = np.zeros((B, S, S), np.float32)
    for c in range(NCORES):
        i0 = c * IPC
        arr = np.asarray(res.results[c]["out"], np.float32).reshape(B, 100, NCOL)
        # pair linear order: col*100 + q -> (i_rel, j') with
        # i_rel = (col*100+q)//200, j' = (col*100+q)%200
        blk = arr.transpose(0, 2, 1).reshape(B, IPC, S)  # [B, i_rel, j']
        out[:, i0: i0 + IPC, perms[c]] = blk
    out *= np.asarray(mask, np.float32)
    if _want_results:
        return out, res
    return out
